# revision 12
# baseline (speedup 1.0000x reference)
"""Trainium2 Bass kernel for per-sample-LoRA causal self-attention (non-causal SDPA).

Sharding: 8 cores = (batch b in 0..3) x (channel-half in 0..1).
Each core computes q/k/v for its 1024 output channels (8 heads) of sample b,
runs attention for those heads, and produces a partial output projection
(contraction over its half of the y channels). Host sums the two partials
per sample and transposes back.

v2: all-bf16 data path (fp8 fails the 2e-2 gate: near-one-hot softmax
columns amplify quantization noise; measured 4-8e-2). LoRA deltas are
merged into the weights on the host (each core owns exactly one sample),
removing the u/uo paths entirely. y stays resident in SBUF.
"""

import os
import sys

sys.path.insert(0, "/opt/trn_rl_repo")

import ml_dtypes
import numpy as np

import concourse.bass as bass  # noqa: F401
import concourse.bass_isa as bass_isa
import concourse.mybir as mybir
import concourse.tile as tile
from concourse import bacc, bass_utils

F32 = mybir.dt.float32
F32R = mybir.dt.float32r
BF16 = mybir.dt.bfloat16
AF = mybir.ActivationFunctionType

B, T, C = 4, 1024, 2048
H, D, R = 16, 128, 16
HALF = C // 2          # output channels per core
HH = HALF // D         # heads per core = 8
CT = C // 128          # contraction tiles over C = 16
IT = HALF // 128       # contraction tiles over half = 8
CH = 512               # t/free chunk
NCH = T // CH          # = 2
PTP = 2                # s_tiles per pT part
SCALE = 1.0 / float(np.sqrt(D))
ROPE_BASE = 10000.0

NP_BF16 = ml_dtypes.bfloat16

_compiled = {}
last_result = None     # BassKernelResults of the most recent run (for test harness)
PHASES = []            # (label, first instruction number) build-time markers


def _mark(nc, label):
    PHASES.append((label, int(nc.get_next_instruction_name().split("-")[1])))


def _build_nc():
    nc = bacc.Bacc("TRN2", target_bir_lowering=False, debug=False, num_devices=8)

    xT = nc.dram_tensor("xT", [C, T], BF16, kind="ExternalInput").ap()
    WqT = nc.dram_tensor("WqT", [C, HALF], BF16, kind="ExternalInput").ap()
    WkT = nc.dram_tensor("WkT", [C, HALF], BF16, kind="ExternalInput").ap()
    WvT = nc.dram_tensor("WvT", [C, HALF], BF16, kind="ExternalInput").ap()
    WoT = nc.dram_tensor("WoT", [HALF, C], BF16, kind="ExternalInput").ap()
    cosT = nc.dram_tensor("cosT", [D, T], F32, kind="ExternalInput").ap()
    sswT = nc.dram_tensor("sswT", [D, T], F32, kind="ExternalInput").ap()
    outT = nc.dram_tensor("outT", [C, T], F32, kind="ExternalOutput").ap()

    with tile.TileContext(nc) as tc:
        with tc.tile_pool(name="tabs", bufs=1) as tabs, \
             tc.tile_pool(name="ps_acc", bufs=3, space="PSUM") as ps_acc, \
             tc.tile_pool(name="ps_s", bufs=3, space="PSUM") as ps_s, \
             tc.tile_pool(name="ps_y", bufs=2, space="PSUM") as ps_y:

            _mark(nc, 'init')
            # ---------------- resident tables ----------------
            x_sb = tabs.tile([128, CT, T], BF16)
            wq_sb = tabs.tile([128, CT, HALF], BF16)
            wk_sb = tabs.tile([128, CT, HALF], BF16)
            v_sb = tabs.tile([128, IT, HALF], BF16)   # [s_in_tile, s_tile, vo]
            y_sb = tabs.tile([128, HH, T], BF16)
            cos_sb = tabs.tile([D, T], F32)
            ssw_sb = tabs.tile([D, T], F32)

            # ============ phase 1: v, per-head qk+attention ============
            # DMA order matters for startup: the first v-psum needs x[ct] and
            # wv[ct] in ct order, so interleave those chunks first and load
            # wq/wk (not needed until qk0, ~60us in) afterwards.
            _mark(nc, 'v')
            with tc.tile_pool(name="wv", bufs=1) as wvp:
                wv_sb = wvp.tile([128, CT, HALF], BF16)
                xr = xT.rearrange("(ct p) t -> p ct t", p=128)
                wvr = WvT.rearrange("(ct p) o -> p ct o", p=128)
                for g in range(8):
                    nc.sync.dma_start(x_sb[:, 2 * g:2 * g + 2, :],
                                      xr[:, 2 * g:2 * g + 2, :])
                    nc.sync.dma_start(wv_sb[:, 2 * g:2 * g + 2, :],
                                      wvr[:, 2 * g:2 * g + 2, :])
                nc.sync.dma_start(cos_sb[:], cosT[:])
                nc.sync.dma_start(ssw_sb[:], sswT[:])
                wqr = WqT.rearrange("(ct p) o -> p ct o", p=128)
                wkr = WkT.rearrange("(ct p) o -> p ct o", p=128)
                for g in range(4):
                    nc.sync.dma_start(wq_sb[:, 4 * g:4 * g + 4, :],
                                      wqr[:, 4 * g:4 * g + 4, :])
                    nc.sync.dma_start(wk_sb[:, 4 * g:4 * g + 4, :],
                                      wkr[:, 4 * g:4 * g + 4, :])
                for ci in range(2):                # vo chunk of 512
                    for tt in range(IT):
                        ps = ps_acc.tile([128, CH], F32, tag="acc")
                        for ct in range(CT):
                            nc.tensor.matmul(ps[:],
                                             x_sb[:, ct, tt * 128:(tt + 1) * 128],
                                             wv_sb[:, ct, ci * CH:(ci + 1) * CH],
                                             start=(ct == 0), stop=(ct == CT - 1))
                        nc.vector.tensor_copy(v_sb[:, tt, ci * CH:(ci + 1) * CH], ps[:])

            # ---- per-head: P1-qk + RoPE + attention ----
            with tc.tile_pool(name="rope", bufs=2) as rope, \
                 tc.tile_pool(name="qk", bufs=3) as qkp, \
                 tc.tile_pool(name="ptp", bufs=5) as ptp, \
                 tc.tile_pool(name="lred", bufs=2) as lred, \
                 tc.tile_pool(name="att", bufs=1) as att:
                for h in range(HH):
                    _mark(nc, f'qk{h}')
                    hs = slice(h * D, (h + 1) * D)
                    rots = []
                    for w_sb in (wq_sb, wk_sb):
                        rot = qkp.tile([D, T], BF16, tag="rot")
                        for ci in range(NCH):
                            cs = slice(ci * CH, (ci + 1) * CH)
                            ps = ps_acc.tile([128, CH], F32, tag="acc")
                            for ct in range(CT):
                                nc.tensor.matmul(ps[:], w_sb[:, ct, hs],
                                                 x_sb[:, ct, cs],
                                                 start=(ct == 0), stop=(ct == CT - 1))
                            # RoPE: rot = q*cos + shift64(q*ssw)
                            t1 = rope.tile([D, CH], BF16, tag="t1")
                            t2 = rope.tile([D, CH], BF16, tag="t2")
                            nc.vector.tensor_mul(t1[:], ps[:], ssw_sb[:, cs])
                            nc.vector.tensor_mul(rot[:, cs], ps[:], cos_sb[:, cs])
                            nc.sync.dma_start(t2[0:64, :], t1[64:128, :])
                            nc.sync.dma_start(t2[64:128, :], t1[0:64, :])
                            nc.vector.tensor_add(rot[:, cs], rot[:, cs], t2[:])
                        rots.append(rot)
                    qr, kr = rots

                    _mark(nc, f'a1_{h}')
                    # A1 + exp -> bf16 pT [128, PTP, T]
                    pts = []
                    for part in range(IT // PTP):
                        pT = ptp.tile([128, PTP, T], BF16, tag="pT")
                        for sp in range(PTP):
                            st = part * PTP + sp
                            for ci in range(NCH):
                                cs = slice(ci * CH, (ci + 1) * CH)
                                ps = ps_s.tile([128, CH], F32, tag="s")
                                nc.tensor.matmul(ps[:],
                                                 kr[:, st * 128:(st + 1) * 128],
                                                 qr[:, cs],
                                                 start=True, stop=True)
                                nc.scalar.activation(pT[:, sp, cs], ps[:],
                                                     AF.Exp, scale=SCALE)
                        pts.append(pT)

                    _mark(nc, f'l_{h}')
                    # l = column sums of p^T on GpSimd; rb = broadcast recip
                    rb = att.tile([128, T], F32, tag="rb")
                    rb1 = att.tile([1, T], F32, tag="rb1")
                    for ci in range(NCH):
                        cs = slice(ci * CH, (ci + 1) * CH)
                        l2 = lred.tile([1, PTP, CH], F32, tag="l2")
                        for g in range(IT // PTP):
                            lr = lred.tile([128, PTP, CH], F32, tag="lr")
                            nc.gpsimd.partition_all_reduce(
                                lr[:], pts[g][:, :, cs], 128,
                                bass_isa.ReduceOp.add)
                            if g == 0:
                                nc.vector.tensor_copy(l2[:], lr[0:1, :, :])
                            else:
                                nc.vector.tensor_add(l2[:], l2[:], lr[0:1, :, :])
                        nc.vector.tensor_add(l2[:, 0, :], l2[:, 0, :], l2[:, 1, :])
                        nc.vector.reciprocal_approx_fast(out=rb1[:, cs],
                                                         in_=l2[:, 0, :])
                        nc.gpsimd.partition_broadcast(rb[:, cs], rb1[:, cs])

                    _mark(nc, f'a2_{h}')
                    # A2 + normalize -> resident y_sb (bf16)
                    for ci in range(NCH):
                        cs = slice(ci * CH, (ci + 1) * CH)
                        yp = ps_y.tile([D, CH], F32, tag="y")
                        for st in range(IT):
                            nc.tensor.matmul(yp[:], v_sb[:, st, hs],
                                             pts[st // PTP][:, st % PTP, cs],
                                             start=(st == 0), stop=(st == IT - 1))
                        nc.vector.tensor_mul(y_sb[:, h, cs], yp[:], rb[:, cs])

            # ============ phase 2: out^T = Wo-half contraction ============
            _mark(nc, 'p2')
            with tc.tile_pool(name="wo", bufs=3) as wop, \
                 tc.tile_pool(name="outp", bufs=3) as outp:
                wor = WoT.rearrange("(it p) o -> p it o", p=128)
                for ot in range(C // 128):
                    wo = wop.tile([128, IT, 128], BF16, tag="wo")
                    nc.sync.dma_start(wo[:, 0:4, :], wor[:, 0:4, ot * 128:(ot + 1) * 128])
                    nc.sync.dma_start(wo[:, 4:8, :], wor[:, 4:8, ot * 128:(ot + 1) * 128])
                    for ci in range(NCH):
                        cs = slice(ci * CH, (ci + 1) * CH)
                        ps = ps_acc.tile([128, CH], F32, tag="acc")
                        for it in range(IT):
                            nc.tensor.matmul(ps[:], wo[:, it, :],
                                             y_sb[:, it, cs],
                                             start=(it == 0), stop=(it == IT - 1))
                        o_sb = outp.tile([128, CH], F32, tag="o")
                        if (ot + ci) % 2 == 0:
                            nc.vector.tensor_copy(o_sb[:], ps[:])
                        else:
                            nc.scalar.activation(o_sb[:], ps[:], AF.Copy)
                        nc.sync.dma_start(outT[ot * 128:(ot + 1) * 128, cs], o_sb[:])

    nc.compile()
    return nc


def _rope_tables():
    inv = (1.0 / (ROPE_BASE ** (np.arange(0, D, 2, dtype=np.float32) / np.float32(D)))).astype(np.float32)
    t_ar = np.arange(T, dtype=np.float32)
    fr = t_ar[:, None] * inv[None, :]
    emb = np.concatenate([fr, fr], axis=1)          # [T, D]
    cos = np.cos(emb).astype(np.float32).T.copy()   # [D, T]
    sin = np.sin(emb).astype(np.float32).T.copy()
    ssw = sin.copy()
    ssw[64:, :] *= -1.0                             # [s; -s] shift-then-add form
    return np.ascontiguousarray(cos), np.ascontiguousarray(ssw)


def _bf(a):
    return np.ascontiguousarray(a, dtype=np.float32).astype(NP_BF16)


def kernel(x, qkvo_delta, Wq, Wk, Wv, Wo):
    global last_result
    x = np.asarray(x, dtype=np.float32)
    qkvo_delta = np.asarray(qkvo_delta, dtype=np.float32)
    Wq = np.asarray(Wq, dtype=np.float32)
    Wk = np.asarray(Wk, dtype=np.float32)
    Wv = np.asarray(Wv, dtype=np.float32)
    Wo = np.asarray(Wo, dtype=np.float32)

    if "nc" not in _compiled:
        _compiled["nc"] = _build_nc()
    nc = _compiled["nc"]

    cos, ssw = _rope_tables()
    d = qkvo_delta.reshape(B, 8, R, C)
    dqA, dqB, dkA, dkB, dvA, dvB, doA, doB = (d[:, i] for i in range(8))

    in_maps = []
    for core in range(8):
        b, half = core // 2, core % 2
        sl = slice(half * HALF, (half + 1) * HALF)
        # merge per-sample LoRA into the weight slices this core owns
        wq_m = Wq[sl, :] + dqB[b][:, sl].T @ dqA[b]      # [HALF, C]
        wk_m = Wk[sl, :] + dkB[b][:, sl].T @ dkA[b]
        wv_m = Wv[sl, :] + dvB[b][:, sl].T @ dvA[b]
        wo_m = Wo[:, sl].T + doA[b][:, sl].T @ doB[b]    # [HALF, C]
        in_maps.append({
            "xT": _bf(x[b].T),
            "WqT": _bf(wq_m.T),
            "WkT": _bf(wk_m.T),
            "WvT": _bf(wv_m.T),
            "WoT": _bf(wo_m),
            "cosT": cos,
            "sswT": ssw,
        })

    trace = bool(int(os.environ.get("KERNEL_TRACE", "0")))
    res = bass_utils.run_bass_kernel_spmd(
        nc, in_maps, core_ids=list(range(8)), trace=trace)
    last_result = res

    out = np.empty((B, T, C), dtype=np.float32)
    for b in range(B):
        acc = res.results[2 * b]["outT"].astype(np.float32) + \
            res.results[2 * b + 1]["outT"].astype(np.float32)
        out[b] = acc.T
    return out
